# revision 6
# baseline (speedup 1.0000x reference)
"""Haar DWT (2x2) Trainium2 Bass kernel — v2 (large-DMA restructure).

Full input x: (8, 64, 512, 512) fp32. Output: tuple (ll, lh, hl, hh), each
(8, 64, 256, 256) fp32.

Sharding: pure data parallel — core i processes batch element i (64, 512, 512).

Per-core algorithm, in 2-channel chunks (32 chunks):
  - One 2 MiB input DMA per chunk loads both channels as X[128, 4096]:
    X[p, (c2*4+rb)*512 + col] = x[c+c2, rb*128+p, col].
  - 8 matmuls (one per 512-col slice) with a constant banded matrix V of
    +-0.5 entries: psum = V^T @ X_slice. Rows 0:64 = 0.5*(row-pair sums),
    rows 64:128 = 0.5*(row-pair diffs) — the vertical Haar stage + scale.
  - Horizontal stage: ACT copies the odd columns PSUM->SBUF (hardware
    allows only one PSUM operand per vector op), then DVE computes
    tlow = even + odd (rows 0:64 = ll, 64:128 = lh) and
    thigh = odd - even (rows 0:64 = hl, 64:128 = hh).
  - 4 output DMAs per chunk (one per output tensor, 512 KiB each).
Input loads issue from the ACT queue, stores from the SP queue, so a store
waiting on compute never blocks the next chunk's load issue.
"""

import sys

if "/opt/trn_rl_repo" not in sys.path:
    sys.path.insert(0, "/opt/trn_rl_repo")

import numpy as np

import concourse.mybir as mybir
from concourse.bacc import Bacc
from concourse.tile import TileContext
from concourse.bass_utils import run_bass_kernel_spmd

N_CORES = 8
C = 64  # images (channels) per core
H = W = 512
OH = OW = 256
CHUNK = 2  # channels per chunk
F32 = mybir.dt.float32

_cache = {}


def build_nc():
    nc = Bacc("TRN2", target_bir_lowering=False, debug=False, num_devices=N_CORES)
    x = nc.declare_dram_parameter("x", [C, H, W], F32, isOutput=False)
    w = nc.declare_dram_parameter("w", [128, 128], F32, isOutput=False)
    ll = nc.declare_dram_parameter("ll", [C, OH, OW], F32, isOutput=True)
    lh = nc.declare_dram_parameter("lh", [C, OH, OW], F32, isOutput=True)
    hl = nc.declare_dram_parameter("hl", [C, OH, OW], F32, isOutput=True)
    hh = nc.declare_dram_parameter("hh", [C, OH, OW], F32, isOutput=True)

    nslice = CHUNK * 4  # 512-col matmul slices per chunk
    with TileContext(nc) as tc:
        with (
            tc.tile_pool(name="const", bufs=1) as cpool,
            tc.tile_pool(name="xin", bufs=3) as xpool,
            tc.tile_pool(name="tl", bufs=3) as tlpool,
            tc.tile_pool(name="th", bufs=3) as thpool,
            tc.tile_pool(name="od", bufs=8) as odpool,
            tc.tile_pool(name="psum", bufs=8, space="PSUM") as ppool,
        ):
            vt = cpool.tile([128, 128], F32)
            nc.sync.dma_start(out=vt, in_=w[:, :])
            for c in range(0, C, CHUNK):
                xt = xpool.tile([128, nslice * W], F32, tag="xt")
                nc.scalar.dma_start(
                    out=xt,
                    in_=x[c : c + CHUNK].rearrange(
                        "c (rb p) w -> p c rb w", rb=4, p=128
                    ),
                )
                tlow = tlpool.tile([128, nslice * OW], F32, tag="tlow")
                thigh = thpool.tile([128, nslice * OW], F32, tag="thigh")
                for k in range(nslice):
                    ps = ppool.tile([128, W], F32, tag="ps")
                    nc.tensor.matmul(
                        out=ps,
                        lhsT=vt,
                        rhs=xt[:, k * W : (k + 1) * W],
                        start=True,
                        stop=True,
                    )
                    # DVE can read only one non-scalar input from PSUM
                    # (NCC_IBVF027), so stage the odd columns through SBUF.
                    od = odpool.tile([128, OW], F32, tag="od")
                    nc.scalar.copy(out=od, in_=ps[:, 1:W:2])
                    nc.vector.tensor_add(
                        out=tlow[:, k * OW : (k + 1) * OW],
                        in0=ps[:, 0:W:2],
                        in1=od,
                    )
                    nc.vector.tensor_sub(
                        out=thigh[:, k * OW : (k + 1) * OW],
                        in0=od,
                        in1=ps[:, 0:W:2],
                    )
                for buf, part, dst in (
                    (tlow, 0, ll),
                    (tlow, 64, lh),
                    (thigh, 0, hl),
                    (thigh, 64, hh),
                ):
                    nc.sync.dma_start(
                        out=dst[c : c + CHUNK].rearrange(
                            "c (rb p) w -> p c rb w", rb=4, p=64
                        ),
                        in_=buf[part : part + 64, :],
                    )
    nc.compile()
    return nc


def make_v():
    v = np.zeros((128, 128), np.float32)
    for m in range(64):
        v[2 * m, m] = 0.5
        v[2 * m + 1, m] = 0.5
        v[2 * m, 64 + m] = -0.5
        v[2 * m + 1, 64 + m] = 0.5
    return v


def get_nc():
    if "nc" not in _cache:
        _cache["nc"] = build_nc()
    return _cache["nc"]


def kernel(x):
    x = np.asarray(x, dtype=np.float32)
    assert x.shape == (N_CORES, C, H, W), x.shape
    nc = get_nc()
    v = make_v()
    in_maps = [{"x": x[i], "w": v} for i in range(N_CORES)]
    res = run_bass_kernel_spmd(nc, in_maps, list(range(N_CORES)))
    outs = []
    for name in ("ll", "lh", "hl", "hh"):
        outs.append(np.stack([res.results[i][name] for i in range(N_CORES)], axis=0))
    return tuple(outs)


# revision 7
# speedup vs baseline: 1.0204x; 1.0204x over previous
"""Haar DWT (2x2) Trainium2 Bass kernel — large-DMA restructure.

Full input x: (8, 64, 512, 512) fp32. Output: tuple (ll, lh, hl, hh), each
(8, 64, 256, 256) fp32.

Sharding: pure data parallel — core i processes batch element i (64, 512, 512).

Per-core algorithm, in channel chunks (mostly 2-channel):
  - One input DMA per chunk loads the chunk as X[128, nslice*512]:
    X[p, (c2*4+rb)*512 + col] = x[c+c2, rb*128+p, col]. Large multi-dim
    DMAs keep the per-instruction issue overhead (~650ns sequencer +
    ~625ns single-slot HWDGE in the cost model) off the critical path.
  - One matmul per 512-col slice with a constant banded matrix V of +-0.5
    entries: psum = V^T @ X_slice. Rows 0:64 = 0.5*(row-pair sums), rows
    64:128 = 0.5*(row-pair diffs) — the vertical Haar stage + scale.
  - Horizontal stage: ACT copies the odd columns PSUM->SBUF (hardware
    allows only one PSUM operand per vector op, NCC_IBVF027), then DVE:
    tlow = even + odd (rows 0:64 = ll, 64:128 = lh),
    thigh = odd - even (rows 0:64 = hl, 64:128 = hh).
  - One store DMA per output tensor per chunk.
Input loads issue from the ACT queue, stores from the SP queue, so a store
waiting on compute never blocks the next chunk's load issue.

Schedule shaping (worth ~8us of DMA-engine idle in the timeline model):
  - The first 3 chunks store in two half-chunk pieces so store traffic
    reaches the DMA engines before the first full chunk finishes (the PE
    runs below peak clock until ~3us of continuous busy, making early
    chunks compute-limited).
  - The last 4 channels are single-channel chunks, shortening the
    compute tail after the final load so the drain stays DMA-busy.
"""

import sys

if "/opt/trn_rl_repo" not in sys.path:
    sys.path.insert(0, "/opt/trn_rl_repo")

import numpy as np

import concourse.mybir as mybir
from concourse.bacc import Bacc
from concourse.tile import TileContext
from concourse.bass_utils import run_bass_kernel_spmd

N_CORES = 8
C = 64  # images (channels) per core
H = W = 512
OH = OW = 256
F32 = mybir.dt.float32

# Channel chunking: 2-channel chunks, except the last 4 channels go one at
# a time (shorter drain). First FIRST_HALF_STORES chunks store in halves.
SIZES = [2] * 30 + [1] * 4
FIRST_HALF_STORES = 3

_cache = {}


def build_nc():
    nc = Bacc("TRN2", target_bir_lowering=False, debug=False, num_devices=N_CORES)
    x = nc.declare_dram_parameter("x", [C, H, W], F32, isOutput=False)
    w = nc.declare_dram_parameter("w", [128, 128], F32, isOutput=False)
    outs = {
        n: nc.declare_dram_parameter(n, [C, OH, OW], F32, isOutput=True)
        for n in ("ll", "lh", "hl", "hh")
    }

    with TileContext(nc) as tc:
        with (
            tc.tile_pool(name="const", bufs=1) as cpool,
            tc.tile_pool(name="xin", bufs=3) as xpool,
            tc.tile_pool(name="tl", bufs=3) as tlpool,
            tc.tile_pool(name="th", bufs=3) as thpool,
            tc.tile_pool(name="od", bufs=8) as odpool,
            tc.tile_pool(name="psum", bufs=8, space="PSUM") as ppool,
        ):
            vt = cpool.tile([128, 128], F32)
            nc.sync.dma_start(out=vt, in_=w[:, :])
            c = 0
            for ci, chunk in enumerate(SIZES):
                nslice = chunk * 4
                xt = xpool.tile([128, nslice * W], F32, tag="xt")
                nc.scalar.dma_start(
                    out=xt,
                    in_=x[c : c + chunk].rearrange(
                        "c (rb p) w -> p c rb w", rb=4, p=128
                    ),
                )
                tlow = tlpool.tile([128, nslice * OW], F32, tag="tlow")
                thigh = thpool.tile([128, nslice * OW], F32, tag="thigh")
                half = ci < FIRST_HALF_STORES and chunk == 2
                for k in range(nslice):
                    ps = ppool.tile([128, W], F32, tag="ps")
                    nc.tensor.matmul(
                        out=ps,
                        lhsT=vt,
                        rhs=xt[:, k * W : (k + 1) * W],
                        start=True,
                        stop=True,
                    )
                    od = odpool.tile([128, OW], F32, tag="od")
                    nc.scalar.copy(out=od, in_=ps[:, 1:W:2])
                    nc.vector.tensor_add(
                        out=tlow[:, k * OW : (k + 1) * OW],
                        in0=ps[:, 0:W:2],
                        in1=od,
                    )
                    nc.vector.tensor_sub(
                        out=thigh[:, k * OW : (k + 1) * OW],
                        in0=od,
                        in1=ps[:, 0:W:2],
                    )
                    if half and k == nslice // 2 - 1:
                        # First half of the chunk (channel c) is done; ship it.
                        for buf, part, dn in (
                            (tlow, 0, "ll"),
                            (tlow, 64, "lh"),
                            (thigh, 0, "hl"),
                            (thigh, 64, "hh"),
                        ):
                            nc.sync.dma_start(
                                out=outs[dn][c : c + 1].rearrange(
                                    "c (rb p) w -> p c rb w", rb=4, p=64
                                ),
                                in_=buf[part : part + 64, 0 : (nslice // 2) * OW],
                            )
                for buf, part, dn in (
                    (tlow, 0, "ll"),
                    (tlow, 64, "lh"),
                    (thigh, 0, "hl"),
                    (thigh, 64, "hh"),
                ):
                    if half:
                        nc.sync.dma_start(
                            out=outs[dn][c + 1 : c + chunk].rearrange(
                                "c (rb p) w -> p c rb w", rb=4, p=64
                            ),
                            in_=buf[part : part + 64, (nslice // 2) * OW :],
                        )
                    else:
                        nc.sync.dma_start(
                            out=outs[dn][c : c + chunk].rearrange(
                                "c (rb p) w -> p c rb w", rb=4, p=64
                            ),
                            in_=buf[part : part + 64, :],
                        )
                c += chunk
    nc.compile()
    return nc


def make_v():
    v = np.zeros((128, 128), np.float32)
    for m in range(64):
        v[2 * m, m] = 0.5
        v[2 * m + 1, m] = 0.5
        v[2 * m, 64 + m] = -0.5
        v[2 * m + 1, 64 + m] = 0.5
    return v


def get_nc():
    if "nc" not in _cache:
        _cache["nc"] = build_nc()
    return _cache["nc"]


def kernel(x):
    x = np.asarray(x, dtype=np.float32)
    assert x.shape == (N_CORES, C, H, W), x.shape
    nc = get_nc()
    v = make_v()
    in_maps = [{"x": x[i], "w": v} for i in range(N_CORES)]
    res = run_bass_kernel_spmd(nc, in_maps, list(range(N_CORES)))
    outs = []
    for name in ("ll", "lh", "hl", "hh"):
        outs.append(np.stack([res.results[i][name] for i in range(N_CORES)], axis=0))
    return tuple(outs)


# revision 8
# speedup vs baseline: 1.0249x; 1.0043x over previous
"""Haar DWT (2x2) Trainium2 Bass kernel — large-DMA restructure.

Full input x: (8, 64, 512, 512) fp32. Output: tuple (ll, lh, hl, hh), each
(8, 64, 256, 256) fp32.

Sharding: pure data parallel — core i processes batch element i (64, 512, 512).

Per-core algorithm, in channel chunks (mostly 2-channel):
  - One input DMA per chunk loads the chunk as X[128, nslice*512]:
    X[p, (c2*4+rb)*512 + col] = x[c+c2, rb*128+p, col]. Large multi-dim
    DMAs keep the per-instruction issue overhead (~650ns sequencer +
    ~625ns single-slot HWDGE in the cost model) off the critical path.
  - One matmul per 512-col slice with a constant banded matrix V of +-0.5
    entries: psum = V^T @ X_slice. Rows 0:64 = 0.5*(row-pair sums), rows
    64:128 = 0.5*(row-pair diffs) — the vertical Haar stage + scale.
  - Horizontal stage: ACT copies the odd columns PSUM->SBUF (hardware
    allows only one PSUM operand per vector op, NCC_IBVF027), then DVE:
    tlow = even + odd (rows 0:64 = ll, 64:128 = lh),
    thigh = odd - even (rows 0:64 = hl, 64:128 = hh).
  - One store DMA per output tensor per chunk.
Input loads issue from the ACT queue, stores from the SP queue, so a store
waiting on compute never blocks the next chunk's load issue.

Schedule shaping (worth ~8us of DMA-engine idle in the timeline model):
  - The first 3 chunks store in two half-chunk pieces so store traffic
    reaches the DMA engines before the first full chunk finishes (the PE
    runs below peak clock until ~3us of continuous busy, making early
    chunks compute-limited).
  - The last 4 channels are single-channel chunks, shortening the
    compute tail after the final load so the drain stays DMA-busy.
"""

import sys

if "/opt/trn_rl_repo" not in sys.path:
    sys.path.insert(0, "/opt/trn_rl_repo")

import numpy as np

import concourse.mybir as mybir
from concourse.bacc import Bacc
from concourse.tile import TileContext
from concourse.bass_utils import run_bass_kernel_spmd

N_CORES = 8
C = 64  # images (channels) per core
H = W = 512
OH = OW = 256
F32 = mybir.dt.float32

# Channel chunking: 2-channel chunks, except the last 4 channels go one at
# a time (shorter drain). First FIRST_HALF_STORES chunks store in halves.
SIZES = [2] * 30 + [1] * 4
FIRST_HALF_STORES = 3

_cache = {}


def build_nc():
    nc = Bacc("TRN2", target_bir_lowering=False, debug=False, num_devices=N_CORES)
    x = nc.declare_dram_parameter("x", [C, H, W], F32, isOutput=False)
    w = nc.declare_dram_parameter("w", [128, 128], F32, isOutput=False)
    outs = {
        n: nc.declare_dram_parameter(n, [C, OH, OW], F32, isOutput=True)
        for n in ("ll", "lh", "hl", "hh")
    }

    with TileContext(nc) as tc:
        with (
            tc.tile_pool(name="const", bufs=1) as cpool,
            tc.tile_pool(name="xin", bufs=3) as xpool,
            tc.tile_pool(name="tl", bufs=3) as tlpool,
            tc.tile_pool(name="th", bufs=3) as thpool,
            tc.tile_pool(name="od", bufs=8) as odpool,
            tc.tile_pool(name="psum", bufs=8, space="PSUM") as ppool,
        ):
            vt = cpool.tile([128, 128], F32)
            nc.sync.dma_start(out=vt, in_=w[:, :])
            c = 0
            for ci, chunk in enumerate(SIZES):
                nslice = chunk * 4
                xt = xpool.tile([128, nslice * W], F32, tag="xt")
                nc.scalar.dma_start(
                    out=xt,
                    in_=x[c : c + chunk].rearrange(
                        "c (rb p) w -> p c rb w", rb=4, p=128
                    ),
                )
                tlow = tlpool.tile([128, nslice * OW], F32, tag="tlow")
                thigh = thpool.tile([128, nslice * OW], F32, tag="thigh")
                half = ci < FIRST_HALF_STORES and chunk == 2
                for k in range(nslice):
                    ps = ppool.tile([128, W], F32, tag="ps")
                    nc.tensor.matmul(
                        out=ps,
                        lhsT=vt,
                        rhs=xt[:, k * W : (k + 1) * W],
                        start=True,
                        stop=True,
                    )
                    od = odpool.tile([128, OW], F32, tag="od")
                    nc.scalar.copy(out=od, in_=ps[:, 1:W:2])
                    # sub before add: hl/hh stores queue behind ll/lh on the
                    # store engine, so producing thigh first removes a
                    # per-chunk stall between the two store pairs.
                    nc.vector.tensor_sub(
                        out=thigh[:, k * OW : (k + 1) * OW],
                        in0=od,
                        in1=ps[:, 0:W:2],
                    )
                    nc.vector.tensor_add(
                        out=tlow[:, k * OW : (k + 1) * OW],
                        in0=ps[:, 0:W:2],
                        in1=od,
                    )
                    if half and k == nslice // 2 - 1:
                        # First half of the chunk (channel c) is done; ship it.
                        for buf, part, dn in (
                            (tlow, 0, "ll"),
                            (tlow, 64, "lh"),
                            (thigh, 0, "hl"),
                            (thigh, 64, "hh"),
                        ):
                            nc.sync.dma_start(
                                out=outs[dn][c : c + 1].rearrange(
                                    "c (rb p) w -> p c rb w", rb=4, p=64
                                ),
                                in_=buf[part : part + 64, 0 : (nslice // 2) * OW],
                            )
                for buf, part, dn in (
                    (tlow, 0, "ll"),
                    (tlow, 64, "lh"),
                    (thigh, 0, "hl"),
                    (thigh, 64, "hh"),
                ):
                    if half:
                        nc.sync.dma_start(
                            out=outs[dn][c + 1 : c + chunk].rearrange(
                                "c (rb p) w -> p c rb w", rb=4, p=64
                            ),
                            in_=buf[part : part + 64, (nslice // 2) * OW :],
                        )
                    else:
                        nc.sync.dma_start(
                            out=outs[dn][c : c + chunk].rearrange(
                                "c (rb p) w -> p c rb w", rb=4, p=64
                            ),
                            in_=buf[part : part + 64, :],
                        )
                c += chunk
    nc.compile()
    return nc


def make_v():
    v = np.zeros((128, 128), np.float32)
    for m in range(64):
        v[2 * m, m] = 0.5
        v[2 * m + 1, m] = 0.5
        v[2 * m, 64 + m] = -0.5
        v[2 * m + 1, 64 + m] = 0.5
    return v


def get_nc():
    if "nc" not in _cache:
        _cache["nc"] = build_nc()
    return _cache["nc"]


def kernel(x):
    x = np.asarray(x, dtype=np.float32)
    assert x.shape == (N_CORES, C, H, W), x.shape
    nc = get_nc()
    v = make_v()
    in_maps = [{"x": x[i], "w": v} for i in range(N_CORES)]
    res = run_bass_kernel_spmd(nc, in_maps, list(range(N_CORES)))
    outs = []
    for name in ("ll", "lh", "hl", "hh"):
        outs.append(np.stack([res.results[i][name] for i in range(N_CORES)], axis=0))
    return tuple(outs)
